# revision 1
# baseline (speedup 1.0000x reference)
"""Cross-cryptocurrency attention kernel for 8 Trainium2 NeuronCores.

Sharding: 16 (batch, seq-quarter) shards -> core c handles b = c//4,
query rows s in [512*(c%4), 512*(c%4+1)).  Each core computes all 8 heads
and all 9 (query-asset, key-asset) pairs for its query slice, with full
keys/values (S=2048) for its batch, so the output projection is local and
no collectives are needed.

v3 design.  ACT exp (75.5M exps/core -> ~572us incl per-inst overhead) is
the hard floor; everything else is organised to hide under it:
  - host folds biases (bk drops out of softmax exactly; bv/bo fold into
    bo2 = bo + (sum_j bv_j) @ Wo) and pre-packs x/weights in bf16 in the
    exact SBUF layouts, so startup DMAs are few and contiguous.
  - x is transposed by the DMA xbar (dma_start_transpose), zero PE cost.
  - all projections run as [128,256]-output chunk units through a
    dedicated psum bank (psD halves, DVE-memset + start=False chains),
    fully decoupled from the score-psum ring so background work never
    stalls the PE->ACT score pipeline.
  - scores^T[t,s] on PE (lhsT=k^T bf16) in groups (2,3,3,3,3,2); exp on
    ACT PSUM->SBUF bf16.  2-tile groups at the combo boundaries cover the
    next combo's catch-up through the 2-deep score ring.
  - AV with the E tile stationary: out[s,33] accumulates over 16 t-tiles
    at 33 rows each (4x less PE than v-stationary); the ones column of
    [v|1] yields row-sums Z in col 32.  Both combo accumulators pack into
    one psum bank (halves, DVE-memset + start=False).
  - normalize straight off the AV psum: DVE reciprocal + STT mul-add.
  - software pipeline: per combo, the last two AV batches + normalize
    defer into the next combo right after its first score group, so ACT
    never waits at combo boundaries; projection/phase-3 units drip at
    four points per combo with deadlines.
"""

import math
import numpy as np

B = 2
S = 2048
D = 256
H = 8
HD = 32
SQ = 512  # query rows per core
N_CORES = 8
SCALE = 1.0 / math.sqrt(HD)
# 2-tile groups at both ends: boundary exps are 1024 elems, long enough to
# cover the next combo's first score group catching up through the ring.
GROUPS = [(0, 2), (2, 3), (5, 3), (8, 3), (11, 3), (14, 2)]

_CACHE = {}


def _build():
    from contextlib import ExitStack

    import concourse.bass as bass
    import concourse.mybir as mybir
    import concourse.tile as tile
    from concourse import bacc
    from concourse.masks import make_identity

    f32 = mybir.dt.float32
    bf16 = mybir.dt.bfloat16
    AF = mybir.ActivationFunctionType
    ALU = mybir.AluOpType

    nc = bacc.Bacc("TRN2", target_bir_lowering=False, debug=False)

    x_d = nc.dram_tensor("x", [3, S, D], bf16, kind="ExternalInput").ap()
    # host-packed weights: [p, (a, ty q/k/v, kt, f)] bf16 and [p, (a,dt)] f32
    wpack_d = nc.dram_tensor("wpack", [128, 3 * 3 * 2 * D], bf16, kind="ExternalInput").ap()
    wo_d = nc.dram_tensor("wo", [128, 2 * D], bf16, kind="ExternalInput").ap()
    bq_d = nc.dram_tensor("bqp", [128, 6], f32, kind="ExternalInput").ap()
    bo2_d = nc.dram_tensor("bo2", [D], bf16, kind="ExternalInput").ap()
    out_d = nc.dram_tensor("out", [3, SQ, D], f32, kind="ExternalOutput").ap()

    with tile.TileContext(nc) as tc, ExitStack() as ctx:
        const_p = ctx.enter_context(tc.tile_pool(name="const", bufs=1))
        qkv_p = ctx.enter_context(tc.tile_pool(name="qkv", bufs=1))
        acc_p = ctx.enter_context(tc.tile_pool(name="acc", bufs=1))
        e_p = ctx.enter_context(tc.tile_pool(name="epool", bufs=4))
        sm_p = ctx.enter_context(tc.tile_pool(name="small", bufs=2))
        # PSUM: 3+3 score ring + 1 packed AV accumulators + 1 drip bank
        ps_S = ctx.enter_context(tc.tile_pool(name="psS", bufs=2, space="PSUM"))
        psAB_p = ctx.enter_context(tc.tile_pool(name="psAB", bufs=1, space="PSUM"))
        psD_p = ctx.enter_context(tc.tile_pool(name="psD", bufs=1, space="PSUM"))
        psAB = psAB_p.tile([128, 512], f32, name="psAB")
        psD = psD_p.tile([128, 512], f32, name="psD")

        xT = [qkv_p.tile([128, 2 * S], bf16, tag=f"xT{_}", name=f"xT{_}") for _ in range(3)]
        kT = [qkv_p.tile([128, 2 * S], bf16, tag=f"kT{_}", name=f"kT{_}") for _ in range(3)]
        qT = [qkv_p.tile([128, 2 * SQ], bf16, tag=f"qT{_}", name=f"qT{_}") for _ in range(3)]
        v1 = [qkv_p.tile([128, 16 * (H * 33)], bf16, tag=f"v1_{_}", name=f"v1_{_}") for _ in range(3)]
        out_acc = [acc_p.tile([128, 4 * D], f32, tag=f"oacc{_}", name=f"oacc{_}") for _ in range(3)]

        def dmaT_unit(a, c):
            def run():
                nc.sync.dma_start_transpose(
                    xT[a].rearrange("p (dt s) -> p dt s", dt=2)[:, :, c * 512 : (c + 1) * 512],
                    x_d[a][c * 512 : (c + 1) * 512, :],
                )
            return run

        # ---- startup DMAs ordered for minimum time-to-first-score-group:
        # SP queue carries x chunk 0 + asset-0 weights; the idle ACT queue
        # dispatches the small/late constants in parallel ----
        dmaT_unit(0, 0)()
        wsb = const_p.tile([128, 3 * 3 * 2 * D], bf16)
        nc.gpsimd.dma_start(wsb[:, 0 : 3 * 2 * D], wpack_d[:, 0 : 3 * 2 * D])
        bq_sb = const_p.tile([128, 6], f32)
        nc.gpsimd.dma_start(bq_sb[:], bq_d)
        for c in range(1, 4):
            dmaT_unit(0, c)()
        nc.sync.dma_start(wsb[:, 3 * 2 * D :], wpack_d[:, 3 * 2 * D :])
        wo_sb = const_p.tile([128, 2 * D], bf16)
        nc.sync.dma_start(wo_sb[:], wo_d)
        bo2_row = const_p.tile([1, D], bf16)
        nc.sync.dma_start(bo2_row[:], bo2_d[None, :])
        ident = const_p.tile([128, 128], f32)
        make_identity(nc, ident[:])
        onesb = const_p.tile([1, 128], bf16)
        nc.gpsimd.memset(onesb[:], 1.0)

        # ---- drip bank: [128,256] halves.  Chains open with start=True:
        # the bank-wide zero region is applied lazily (zero-on-next-matmul-
        # touch), so the other half's finished result stays readable for its
        # pending DVE copy; chains themselves are sequential in PE order. ----
        dctr = [0]

        def dhalf():
            hh = dctr[0] & 1
            dctr[0] += 1
            return psD[:, hh * 256 : (hh + 1) * 256]

        WT_Q, WT_K, WT_V = 0, 1, 2

        def wcol(a, ty, kt, off, width):
            base = a * (3 * 2 * D) + ty * (2 * D) + kt * D + off
            return wsb[:, base : base + width]

        def k_unit(a, dt, n):  # kT[a] cols [dt*S + 256n, +256)
            def run():
                reg = dhalf()
                for kt in range(2):
                    nc.tensor.matmul(
                        reg,
                        wcol(a, WT_K, kt, dt * 128, 128),
                        xT[a][:, kt * S + n * 256 : kt * S + (n + 1) * 256],
                        start=(kt == 0), stop=(kt == 1), skip_group_check=True,
                    )
                nc.vector.tensor_copy(
                    kT[a][:, dt * S + n * 256 : dt * S + (n + 1) * 256], reg
                )
            return run

        def q_unit(a, dt, n):  # qT[a] cols [dt*SQ + 256n, +256)
            def run():
                reg = dhalf()
                for kt in range(2):
                    nc.tensor.matmul(
                        reg,
                        wcol(a, WT_Q, kt, dt * 128, 128),
                        xT[a][:, kt * S + n * 256 : kt * S + (n + 1) * 256],
                        start=(kt == 0), stop=(kt == 1), skip_group_check=True,
                    )
                nc.vector.tensor_scalar_add(
                    qT[a][:, dt * SQ + n * 256 : dt * SQ + (n + 1) * 256],
                    reg,
                    bq_sb[:, a * 2 + dt : a * 2 + dt + 1],
                )
            return run

        def v_unit(a, st):
            def run():
                reg = dhalf()
                for kt in range(2):
                    nc.tensor.matmul(
                        reg,
                        xT[a][:, kt * S + st * 128 : kt * S + (st + 1) * 128],
                        wcol(a, WT_V, kt, 0, D),
                        start=(kt == 0), stop=(kt == 1), skip_group_check=True,
                    )
                dst = v1[a][
                    :, st * (H * 33) : (st + 1) * (H * 33)
                ].rearrange("p (h x) -> p h x", x=33)[:, :, 0:32]
                nc.vector.tensor_copy(dst, reg.rearrange("p (h x) -> p h x", x=32))
            return run

        def ones_unit(a):
            def run():
                nc.gpsimd.memset(
                    v1[a].rearrange("p (t h x) -> p (t h) x", h=H, x=33)[:, :, 32:33],
                    1.0,
                )
            return run

        # ======== Phase 2: one (i, j, h) combo ========
        def emit_av(eg, t0, glen, j, h, reg):
            for u in range(glen):
                tt = t0 + u
                for k in range(4):
                    nc.tensor.matmul(
                        reg[:, k * 33 : (k + 1) * 33],
                        eg[:, u * 512 + k * 128 : u * 512 + (k + 1) * 128],
                        v1[j][:, tt * (H * 33) + h * 33 : tt * (H * 33) + (h + 1) * 33],
                        start=False,
                        stop=(tt == 15 and k == 3),
                        skip_group_check=True,
                    )

        def norm_unit(i, j, h, reg):
            def run():
                rr4 = sm_p.tile([128, 4], f32, tag="rr", name="rr")
                nc.vector.reciprocal_approx_fast(
                    rr4[:],
                    reg.rearrange("p (k x) -> p k x", x=33)[:, :, 32],
                )
                for k in range(4):
                    oa = out_acc[i][:, k * D + h * 32 : k * D + (h + 1) * 32]
                    src = reg[:, k * 33 : k * 33 + 32]
                    if j == 0:
                        nc.vector.tensor_scalar_mul(oa, src, rr4[:, k : k + 1])
                    else:
                        nc.vector.scalar_tensor_tensor(
                            oa, src, rr4[:, k : k + 1], oa, op0=ALU.mult, op1=ALU.add
                        )
            return run

        def combo(ci, i, j, h, tail, drip, pre_sc=None, pre_av=None, pre_tail=None):
            hp = 32 * (h % 4)
            hc = h // 4
            reg = psAB[:, (ci % 2) * 256 : (ci % 2) * 256 + 132]
            nc.vector.memset(reg, 0.0)
            egs = []

            def sc(gi):
                t0, glen = GROUPS[gi]
                psS = ps_S.tile([128, glen * 512], f32, tag="psS", name="ps2")
                for u in range(glen):
                    tt = t0 + u
                    nc.tensor.matmul(
                        psS[:, u * 512 : (u + 1) * 512],
                        kT[j][hp : hp + 32, hc * S + tt * 128 : hc * S + (tt + 1) * 128],
                        qT[i][hp : hp + 32, hc * SQ : (hc + 1) * SQ],
                        start=True,
                        stop=True,
                        tile_position=(hp, 0),
                    )
                eg = e_p.tile([128, 3 * 512], bf16, tag="eg", name="eg")
                nc.scalar.activation(eg[:, 0 : glen * 512], psS[:], AF.Exp, scale=SCALE)
                egs.append((eg, t0, glen))

            def hook(d, gi):
                if d and gi in d:
                    for u in d[gi]:
                        u()

            hook(pre_sc, 0)
            sc(0)
            if pre_tail:
                for u in pre_tail:
                    u()
            for t in tail:
                t()
            hook(pre_sc, 1)
            drip(1)
            sc(1)
            hook(pre_av, 0)
            drip(1)
            emit_av(*egs[0], j, h, reg)
            hook(pre_sc, 2)
            drip(1)
            sc(2)
            hook(pre_av, 1)
            drip(1)
            emit_av(*egs[1], j, h, reg)
            hook(pre_sc, 3)
            drip(1)
            sc(3)
            hook(pre_av, 2)
            drip(1)
            emit_av(*egs[2], j, h, reg)
            hook(pre_sc, 4)
            drip(1)
            sc(4)
            hook(pre_av, 3)
            drip(1)
            emit_av(*egs[3], j, h, reg)
            hook(pre_sc, 5)
            drip(1)
            sc(5)
            return [
                lambda: emit_av(*egs[4], j, h, reg),
                lambda: emit_av(*egs[5], j, h, reg),
                norm_unit(i, j, h, reg),
            ]

        # ======== Phase 3: output projection for one asset, as units ======
        aT = [acc_p.tile([128, 2 * SQ], bf16, tag=f"aT{_}", name=f"aT{_}") for _ in range(3)]

        def t_unit(i, dt, half):  # transpose out_acc block -> aT bf16
            def run():
                reg = dhalf()
                for k in range(2):
                    st = 2 * half + k
                    nc.tensor.matmul(
                        reg[:, k * 128 : (k + 1) * 128],
                        out_acc[i][:, st * D + dt * 128 : st * D + dt * 128 + 128],
                        ident[:],
                        is_transpose=True,
                        start=(k == 0), stop=True, skip_group_check=True,
                    )
                nc.vector.tensor_copy(
                    aT[i][:, dt * SQ + half * 256 : dt * SQ + (half + 1) * 256], reg
                )
            return run

        def p_unit(i, st):
            def run():
                reg = dhalf()
                for dt in range(2):
                    nc.tensor.matmul(
                        reg,
                        aT[i][:, dt * SQ + st * 128 : dt * SQ + (st + 1) * 128],
                        wo_sb[:, dt * D : (dt + 1) * D],
                        start=(dt == 0), stop=False, skip_group_check=True,
                    )
                nc.tensor.matmul(
                    reg,
                    onesb[0:1, 0:128],
                    bo2_row[0:1, :],
                    start=False, stop=True, skip_group_check=True,
                )
                ot = sm_p.tile([128, D], f32, tag="ot", name="ot")
                nc.vector.tensor_copy(ot[:], reg)
                nc.sync.dma_start(
                    out_d[i].rearrange("(st p) d -> st p d", p=128)[st], ot[:]
                )
            return run

        def ph3_units(i):
            return [t_unit(i, dt, half) for dt in range(2) for half in range(2)] + [
                p_unit(i, st) for st in range(4)
            ]

        # ======== Emission schedule ========
        # startup prefix: just enough for combo (0,0,0) group 0
        q_unit(0, 0, 0)()
        q_unit(0, 0, 1)()
        k_unit(0, 0, 0)()
        ones_unit(0)()

        # combo-0/1 custom placement: asset-0 dt0 k-units and v-units land
        # exactly where the score groups / AV batches need them
        c0_pre_sc = {
            1: [k_unit(0, 0, 1), k_unit(0, 0, 2)],
            2: [k_unit(0, 0, 3)],
            3: [k_unit(0, 0, 4), k_unit(0, 0, 5)],
            4: [k_unit(0, 0, 6)],
            5: [k_unit(0, 0, 7)],
        }
        c0_pre_av = {
            0: [v_unit(0, 0), v_unit(0, 1)],
            1: [v_unit(0, 2), v_unit(0, 3), v_unit(0, 4)],
            2: [v_unit(0, 5), v_unit(0, 6), v_unit(0, 7)],
            3: [v_unit(0, 8), v_unit(0, 9), v_unit(0, 10)],
        }
        c1_pre_tail = [v_unit(0, st) for st in range(11, 16)]

        # deadline-tagged drip queue
        dripq = []
        for dt in range(2):
            for n in range(2):
                dripq.append((q_unit(0, 1, n), 0))
        for n in range(8):
            dripq.append((k_unit(0, 1, n), 0))
        for a in (1, 2):
            for c in range(4):
                dripq.append((dmaT_unit(a, c), 0))
            dripq.append((ones_unit(a), 0))
            for dt in range(2):
                for n in range(2):
                    dripq.append((q_unit(a, dt, n), 0))
            for dt in range(2):
                for n in range(8):
                    dripq.append((k_unit(a, dt, n), 0))
            for st in range(16):
                dripq.append((v_unit(a, st), 0))
        for u in ph3_units(0):
            dripq.append((u, 24))
        for u in ph3_units(1):
            dripq.append((u, 48))

        ci_box = [0]

        def drip(budget):
            while dripq and budget > 0 and dripq[0][1] <= ci_box[0]:
                dripq.pop(0)[0]()
                budget -= 1

        tail = []
        ci = 0
        for i in range(3):
            for j in range(3):
                for h in range(H):
                    ci_box[0] = ci
                    combo_kw = {}
                    if ci == 0:
                        combo_kw = dict(pre_sc=c0_pre_sc, pre_av=c0_pre_av)
                    elif ci == 1:
                        combo_kw = dict(pre_tail=c1_pre_tail)
                    tail = combo(ci, i, j, h, tail, drip, **combo_kw)
                    ci += 1
        for t in tail:
            t()
        while dripq:
            dripq.pop(0)[0]()
        for u in ph3_units(2):
            u()
    nc.compile()
    return nc


def kernel(x_btc, x_eth, x_sol, Wq, bq, Wk, bk, Wv, bv, Wo, bo):
    import ml_dtypes
    from concourse.bass_utils import run_bass_kernel_spmd

    if "nc" not in _CACHE:
        _CACHE["nc"] = _build()
    nc = _CACHE["nc"]

    bff = ml_dtypes.bfloat16
    xs = [np.asarray(t, dtype=np.float32) for t in (x_btc, x_eth, x_sol)]
    # fold v-bias and o-bias: out = attn @ Wo + (sum_j bv_j) @ Wo + bo
    bo2 = (np.asarray(bo, np.float64)
           + np.asarray(bv, np.float64).sum(0) @ np.asarray(Wo, np.float64))
    # weight pack [p, (a, ty, kt, f)]: wpack[p, a,ty,kt,f] = W_ty[a, kt*128+p, f]
    wqkv = np.stack([np.asarray(W, np.float32) for W in (Wq, Wk, Wv)], axis=1)
    wpack = np.ascontiguousarray(
        wqkv.reshape(3, 3, 2, 128, D).transpose(3, 0, 1, 2, 4).reshape(128, 3 * 3 * 2 * D)
    ).astype(bff)
    wo_p = np.ascontiguousarray(
        np.asarray(Wo, np.float32).reshape(2, 128, D).transpose(1, 0, 2).reshape(128, 2 * D)
    ).astype(bff)
    bq_p = np.ascontiguousarray(
        np.asarray(bq, np.float32).reshape(3, 2, 128).transpose(2, 0, 1).reshape(128, 6)
    )
    common = {
        "wpack": wpack,
        "wo": wo_p,
        "bqp": bq_p,
        "bo2": bo2.astype(np.float32).astype(bff),
    }
    in_maps = []
    for c in range(N_CORES):
        b, sq = c // 4, c % 4
        # Roll the sequence so this core's query quarter sits at rows [0:512)
        # (the kernel always projects q from rows 0:512).  k/v see the rolled
        # full sequence, which is fine: softmax+sum over the key axis is
        # permutation-invariant.
        xq = np.stack(
            [np.roll(xs[i][b], -sq * SQ, axis=0) for i in range(3)]
        ).astype(bff)
        in_maps.append({"x": np.ascontiguousarray(xq), **common})
    import os
    res = run_bass_kernel_spmd(
        nc, in_maps, core_ids=list(range(N_CORES)),
        trace=bool(os.environ.get("BASS_TRACE")),
    )
    _CACHE["last_res"] = res

    outs = [np.empty((B, S, D), np.float32) for _ in range(3)]
    for c in range(N_CORES):
        b, sq = c // 4, c % 4
        o = res.results[c]["out"]
        for i in range(3):
            outs[i][b, sq * SQ : (sq + 1) * SQ] = o[i]
    return tuple(outs)


if __name__ == "__main__":
    import reference

    inp = reference.setup_inputs()
    inp = {k: np.asarray(v) for k, v in inp.items()}
    got = kernel(**inp)
    exp = reference.reference(**inp)
    for i in range(3):
        g, e = np.asarray(got[i]), np.asarray(exp[i])
        err = np.abs(g - e).max() / np.abs(e).max()
        print(f"out[{i}] rel err {err:.3e}")

